# revision 1
# baseline (speedup 1.0000x reference)
"""Self-contained Trainium2 Bass kernel: GRU(relu, reset_after) + BN + Dense.

kernel(**inputs) takes FULL unsharded fp32 inputs, shards batch over 8
NeuronCores, runs the Bass kernel via run_bass_kernel_spmd, returns the
FULL [2048, 1] fp32 output.

v2.1 layout (per core):
  B=256 batch (2 chunks of Bc=128), T=256 steps, F=32 in-feats, H=256 hidden.
  Transposed: H on partitions (2 H-blocks as column blocks), batch on free dim.

Design:
  - Recurrent matmuls in fp8e4 DoubleRow mode: contraction K=256 packed
    2-per-partition, 6 matmuls/chunk/step, 0.5 cyc/row.
  - Biases (b_z, b_r, b_xh) folded into the x-projection via an augmented
    33rd constant-1 input row; b_rh added by a single K=2 one-hot matmul.
  - PSUM banks paired [r|xh] and [z|rh] per chunk so the r-group closes right
    after the 2 recurrent r-matmuls -> sigmoid(r) starts early.
  - Post-relu tail (d, e, hn8) merged across both chunks into 512-col DVE ops
    on shared tiles; fp16 h copy produced on GpSimd.

DRAM inputs (host-prepped):
  xT   [T*64, 256] f16  row (t//2)*128 + 64*(t%2) + f = x feat f at step t
                        (f=32 row is the constant 1.0), col = batch
  wi   [128, 768]  f16  rows 64g+0:32 input kernel, row 64g+32 = bias row
  wr   [128, 1536] f8e4 col m*256+i*128+j = rec[i*128+p, m*128+j]
  brh2 [2, 128]    f16  b_rh halves (lhsT of the one-hot bias matmul)
  oh2  [2, 256]    f16  one-hot rows selecting H-block column ranges
  sv   [128, 2]    f16  BN+dense folded scale, col c = s[c*128:(c+1)*128]
  cv   [1, 1]      f32  scalar constant folded from BN/dense biases
Output:
  y    [1, 256] f32  per-core output slice (before host concat)
"""
from contextlib import ExitStack

import numpy as np

import concourse.bass as bass
import concourse.tile as tile
from concourse import bacc, mybir

F16 = mybir.dt.float16
F32 = mybir.dt.float32
F8E4 = mybir.dt.float8e4
AF = mybir.ActivationFunctionType
DR = mybir.MatmulPerfMode.DoubleRow


def build_gru_nc(T=256, debug=False):
    nc = bacc.Bacc("TRN2", num_devices=8, debug=debug)
    xT_d = nc.dram_tensor("xT", [T * 64, 256], F16, kind="ExternalInput")
    wi_d = nc.dram_tensor("wi", [128, 768], F16, kind="ExternalInput")
    wr_d = nc.dram_tensor("wr", [128, 1536], F8E4, kind="ExternalInput")
    brh2_d = nc.dram_tensor("brh2", [2, 128], F16, kind="ExternalInput")
    oh2_d = nc.dram_tensor("oh2", [2, 256], F16, kind="ExternalInput")
    sv_d = nc.dram_tensor("sv", [128, 2], F16, kind="ExternalInput")
    cv_d = nc.dram_tensor("cv", [1, 1], F32, kind="ExternalInput")
    y_d = nc.dram_tensor("y", [1, 256], F32, kind="ExternalOutput")

    with tile.TileContext(nc) as tc, ExitStack() as ctx:
        const = ctx.enter_context(tc.tile_pool(name="const", bufs=1))
        hcp = ctx.enter_context(tc.tile_pool(name="hc", bufs=4))
        h8p = ctx.enter_context(tc.tile_pool(name="h8c", bufs=4))
        gpool = [
            ctx.enter_context(tc.tile_pool(name=f"g{c}", bufs=4)) for c in (0, 1)
        ]
        # bank A = [r | xh], bank B = [z | rh], per chunk
        rxp = [
            ctx.enter_context(
                tc.tile_pool(name=f"rx{c}", bufs=2, space=bass.MemorySpace.PSUM)
            )
            for c in (0, 1)
        ]
        zrhp = [
            ctx.enter_context(
                tc.tile_pool(name=f"zrh{c}", bufs=2, space=bass.MemorySpace.PSUM)
            )
            for c in (0, 1)
        ]

        # ---- constants / weights ----
        xsb = const.tile([128, (T // 2) * 256], F16)  # x tiles, col blk j=t//2
        wi = const.tile([128, 768], F16)
        wr = const.tile([128, 1536], F8E4)
        brh2 = const.tile([2, 128], F16)
        oh2 = const.tile([2, 256], F16)
        sv = const.tile([128, 2], F16)
        cv = const.tile([1, 1], F32)

        nc.sync.dma_start(wi[:], wi_d.ap())
        nc.sync.dma_start(wr[:], wr_d.ap())
        nc.sync.dma_start(brh2[:], brh2_d.ap())
        nc.sync.dma_start(oh2[:], oh2_d.ap())
        nc.sync.dma_start(sv[:], sv_d.ap())
        nc.sync.dma_start(cv[:], cv_d.ap())

        ntile = T // 2
        nchunk = 4
        per = ntile // nchunk
        for jc in range(nchunk):
            src = xT_d.ap()[jc * per * 128 : (jc + 1) * per * 128, :]
            src = src.rearrange("(j p) b -> p j b", p=128)
            dst = xsb[:, jc * per * 256 : (jc + 1) * per * 256]
            dst = dst.rearrange("p (j b) -> p j b", b=256)
            nc.sync.dma_start(dst, src)

        # ---- initial hidden state (shared combined tiles, col = c*256+blk*128+b)
        h16 = hcp.tile([128, 512], F16)
        nc.vector.memset(h16[:], 0.0)
        h8 = h8p.tile([128, 512], F8E4)
        nc.vector.memset(h8[:], 0.0)

        def x_phase(t, c):
            """x-projection (+biases) for step t, chunk c.
            rx bank: r (0:256) | xh (256:512);  zrh bank: z (0:256) | rh (256:512)
            """
            rx = rxp[c].tile([128, 512], F32)
            zrh = zrhp[c].tile([128, 512], F32)
            g = t % 2
            col0 = (t // 2) * 256 + c * 128
            xrhs = xsb[64 * g : 64 * g + 33, col0 : col0 + 128]
            # wi col order: z(0:256) r(256:512) xh(512:768); m = gate*2+blk
            # r blocks -> rx[0:256]
            for blk in (0, 1):
                lhsT = wi[64 * g : 64 * g + 33, 256 + blk * 128 : 256 + (blk + 1) * 128]
                nc.tensor.matmul(
                    rx[:, blk * 128 : (blk + 1) * 128], lhsT, xrhs,
                    start=(blk == 0), stop=False,
                    tile_position=(64 * g, 0), skip_group_check=True,
                )
            # xh blocks -> rx[256:512]
            for blk in (0, 1):
                lhsT = wi[64 * g : 64 * g + 33, 512 + blk * 128 : 512 + (blk + 1) * 128]
                nc.tensor.matmul(
                    rx[:, 256 + blk * 128 : 256 + (blk + 1) * 128], lhsT, xrhs,
                    start=False, stop=False,
                    tile_position=(64 * g, 0), skip_group_check=True,
                )
            # z blocks -> zrh[0:256]
            for blk in (0, 1):
                lhsT = wi[64 * g : 64 * g + 33, blk * 128 : (blk + 1) * 128]
                nc.tensor.matmul(
                    zrh[:, blk * 128 : (blk + 1) * 128], lhsT, xrhs,
                    start=(blk == 0), stop=False,
                    tile_position=(64 * g, 0), skip_group_check=True,
                )
            # b_rh -> zrh[256:512] via one-hot K=2 matmul
            nc.tensor.matmul(
                zrh[:, 256:512], brh2[0:2, :], oh2[0:2, :],
                start=False, stop=False, tile_position=(0, 0),
                skip_group_check=True,
            )
            return rx, zrh

        cur = [x_phase(0, 0), x_phase(0, 1)]

        ecomb = []
        for t in range(T):
            hn8 = h8p.tile([128, 512], F8E4)
            hn16 = hcp.tile([128, 512], F16)
            for c in (0, 1):
                rx, zrh = cur[c]
                rhs3 = h8[:, c * 256 : (c + 1) * 256].rearrange(
                    "p (i b) -> p i b", i=2
                )
                # rec matmuls fp8 DoubleRow: r first (closes rx bank), then
                # rh, then z (closes zrh bank)
                for m in (2, 3, 4, 5, 0, 1):
                    lhsT3 = wr[:, m * 256 : (m + 1) * 256].rearrange(
                        "p (i j) -> p i j", i=2
                    )
                    if m in (2, 3):        # r -> rx[0:256]
                        out = rx[:, (m - 2) * 128 : (m - 1) * 128]
                    elif m in (4, 5):      # rh -> zrh[256:512]
                        out = zrh[:, 256 + (m - 4) * 128 : 256 + (m - 3) * 128]
                    else:                  # z -> zrh[0:256]
                        out = zrh[:, m * 128 : (m + 1) * 128]
                    nc.tensor.matmul(
                        out, lhsT3, rhs3, start=False, stop=(m in (3, 1)),
                        perf_mode=DR, skip_group_check=True,
                    )

                r_sb = gpool[c].tile([128, 256], F16, tag="r")
                nc.scalar.activation(r_sb[:], rx[:, 0:256], AF.Sigmoid)
                z_sb = gpool[c].tile([128, 256], F16, tag="z")
                nc.scalar.activation(z_sb[:], zrh[:, 0:256], AF.Sigmoid)

                p = gpool[c].tile([128, 256], F16, tag="p")
                nc.vector.tensor_mul(p[:], r_sb[:], zrh[:, 256:512])
                pre = gpool[c].tile([128, 256], F16, tag="pre")
                nc.vector.tensor_add(pre[:], rx[:, 256:512], p[:])
                hh = gpool[c].tile([128, 256], F16, tag="hh")
                nc.vector.tensor_scalar_max(hh[:], pre[:], 0.0)
                d = gpool[c].tile([128, 256], F16, tag="d")
                nc.vector.tensor_sub(d[:], h16[:, c * 256 : (c + 1) * 256], hh[:])
                e = gpool[c].tile([128, 256], F16, tag="e")
                nc.vector.tensor_mul(e[:], z_sb[:], d[:])
                nc.vector.tensor_add(hn8[:, c * 256 : (c + 1) * 256], hh[:], e[:])
                ecomb.append((hh, e))

                if t + 1 < T:
                    cur[c] = x_phase(t + 1, c)
            for c in (0, 1):
                hh, e = ecomb[c]
                nc.gpsimd.tensor_add(
                    hn16[:, c * 256 : (c + 1) * 256], hh[:], e[:]
                )
            ecomb.clear()
            h8 = hn8
            h16 = hn16

        # ---- BN + dense epilogue: y = s . h + c ----
        fin = rxp[0].tile([128, 512], F32, name="rx")
        first = True
        for c in (0, 1):
            for ct in (0, 1):
                nc.tensor.matmul(
                    fin[0:1, c * 128 : (c + 1) * 128],
                    sv[:, ct : ct + 1],
                    h16[:, c * 256 + ct * 128 : c * 256 + (ct + 1) * 128],
                    start=first,
                    stop=(c == 1 and ct == 1),
                )
                first = False
        ysb = const.tile([1, 256], F32)
        nc.vector.tensor_scalar_add(ysb[:], fin[0:1, 0:256], cv[0:1, 0:1])
        nc.sync.dma_start(y_d.ap(), ysb[:])

    nc.compile()
    return nc


BN_EPS = 1e-3


def prep_core_inputs(x_core, kernel, rec_kernel, bias, gamma, beta,
                     moving_mean, moving_var, dense_w, dense_b):
    """Host-side prep of one core's input dict. x_core: [B=256, T, 32] f32."""
    import ml_dtypes
    B, T, F = x_core.shape
    H = 256
    xT = np.zeros((T // 2, 128, B), np.float16)
    for g in (0, 1):
        xg = x_core[:, g::2, :].astype(np.float16)      # [B, T/2, 32]
        xT[:, 64 * g : 64 * g + 32, :] = xg.transpose(1, 2, 0)
        xT[:, 64 * g + 32, :] = 1.0
    xT = np.ascontiguousarray(xT.reshape(T * 64, B))

    b_z = bias[0, 0:256] + bias[1, 0:256]
    b_r = bias[0, 256:512] + bias[1, 256:512]
    b_xh = bias[0, 512:768]
    b_rh = bias[1, 512:768]
    bias_row = np.concatenate([b_z, b_r, b_xh])
    wi = np.zeros((128, 768), np.float16)
    for g in (0, 1):
        wi[64 * g : 64 * g + 32, :] = kernel.astype(np.float16)
        wi[64 * g + 32, :] = bias_row.astype(np.float16)

    rec8 = np.asarray(rec_kernel, dtype=ml_dtypes.float8_e4m3)
    wr = np.zeros((128, 1536), ml_dtypes.float8_e4m3)
    for m in range(6):
        for i in (0, 1):
            wr[:, m * 256 + i * 128 : m * 256 + (i + 1) * 128] = (
                rec8[i * 128 : (i + 1) * 128, m * 128 : (m + 1) * 128]
            )

    brh2 = np.stack([b_rh[:128], b_rh[128:]], axis=0).astype(np.float16)
    oh2 = np.zeros((2, 256), np.float16)
    oh2[0, 0:128] = 1.0
    oh2[1, 128:256] = 1.0

    rs = 1.0 / np.sqrt(moving_var + BN_EPS)
    s = (gamma * rs * dense_w[:, 0]).astype(np.float16)
    sv = np.stack([s[:128], s[128:]], axis=1)
    cc = dense_b[0] + np.sum((beta - moving_mean * gamma * rs) * dense_w[:, 0])
    cv = np.array([[cc]], np.float32)
    return {
        "xT": xT,
        "wi": np.ascontiguousarray(wi),
        "wr": np.ascontiguousarray(wr),
        "brh2": np.ascontiguousarray(brh2),
        "oh2": oh2,
        "sv": np.ascontiguousarray(sv),
        "cv": cv,
    }


_NC_CACHE = {}


def _get_nc():
    if "nc" not in _NC_CACHE:
        _NC_CACHE["nc"] = build_gru_nc(T=256)
    return _NC_CACHE["nc"]


def kernel(x, kernel, rec_kernel, bias, gamma, beta, moving_mean, moving_var,
           dense_w, dense_b):
    from concourse.bass_utils import run_bass_kernel_spmd

    x = np.asarray(x, dtype=np.float32)
    args = [np.asarray(a, dtype=np.float32) for a in
            (kernel, rec_kernel, bias, gamma, beta, moving_mean, moving_var,
             dense_w, dense_b)]
    nc = _get_nc()
    n_cores = 8
    nb = x.shape[0] // n_cores
    in_maps = [prep_core_inputs(x[i * nb : (i + 1) * nb], *args)
               for i in range(n_cores)]
    res = run_bass_kernel_spmd(nc, in_maps, core_ids=list(range(n_cores)))
    return np.concatenate(
        [res.results[i]["y"].reshape(nb, 1) for i in range(n_cores)], axis=0
    ).astype(np.float32)

